# revision 1
# baseline (speedup 1.0000x reference)
"""Trainium2 Bass kernel for nn_CompLayer_37512244363763 (GNN message passing).

Strategy: dst-range sharding over 8 cores (no collectives). Each core owns
nodes [c*5000, (c+1)*5000). Host pre-sorts edges by (dst-block, src-range),
pads each (block, src-half) section to 128-edge chunks with shared-across-core
capacities so one SPMD program serves all cores.

Per 128-edge chunk (all edges target one 125-node block):
  hs = ent[src], rl = rel_emb[rel_id], hd = ent[dst]   (dma_gather, int16 idx)
  comp = hs*rl ; score = rowsum(comp*hd) ; ex = exp(score)
  A[e,n] = (dst_local[e]==n) * ex[e]                    (one tensor_scalar op)
  psum[125,129] += A^T @ [comp | 1]                     (PE accumulate)
Block epilogue: neigh = psum[:,:128]/(psum[:,128]+eps); out = tanh(neigh @ W)
computed transposed so the store needs no DMA transpose; host transposes back.
"""
import numpy as np

import concourse.bacc as bacc
import concourse.bass as bass
import concourse.tile as tile
import concourse.mybir as mybir
from concourse.bass_utils import run_bass_kernel_spmd
from concourse.masks import make_identity

f32 = mybir.dt.float32
i16 = mybir.dt.int16

N = 40000
E = 640000
D = 128
R2 = 474
NCORES = 8
NS = N // NCORES          # nodes per core
BLK = 125                 # nodes per PSUM block
NBLK = NS // BLK          # blocks per core
P = 128                   # edges per chunk
SB = 2                    # blocks per gather super
SPLIT = 32768             # int16 index limit for src gathers


def _ceil128(x):
    return ((x + 127) // 128) * 128


def build_layout(ent, rel_emb, neigh_w, src, dst, rel_id, split=SPLIT,
                 n=N, d=D, ncores=NCORES, blk=BLK):
    """Host-side shard + sort + pad. Returns (meta, in_maps)."""
    ns = n // ncores
    nblk = ns // blk
    e = src.shape[0]

    gb = dst.astype(np.int64) // blk                   # global block id
    half = (src >= split).astype(np.int64)
    key = gb * 2 + half
    order = np.argsort(key, kind="stable")
    nsec_g = ncores * nblk * 2
    cnt = np.bincount(key, minlength=nsec_g)
    Lc = cnt[0::2].reshape(ncores, nblk)
    Hc = cnt[1::2].reshape(ncores, nblk)
    capL = _ceil128(Lc.max(axis=0))
    capH = _ceil128(Hc.max(axis=0))
    empty = (capL + capH) == 0
    capL[empty] = 128
    n_chunks = (capL + capH) // 128                    # per in-core block
    c_total = int(n_chunks.sum())
    totslot = 128 * c_total

    blockstart = np.zeros(nblk + 1, np.int64)
    blockstart[1:] = np.cumsum(capL + capH)
    # per-core-local section starts, indexed by (b_incore*2 + half)
    secstart = np.zeros(nblk * 2, np.int64)
    secstart[0::2] = blockstart[:-1]
    secstart[1::2] = blockstart[:-1] + capL

    # per-edge slot within its core's slot space
    gfirst = np.zeros(nsec_g, np.int64)
    gfirst[1:] = np.cumsum(cnt)[:-1]
    ranks = np.arange(e, dtype=np.int64) - np.repeat(gfirst, cnt)
    key_sorted = key[order]
    sec_local = (key_sorted % (nblk * 2))
    slot_sorted = secstart[sec_local] + ranks
    core_sorted = key_sorted // (nblk * 2)

    src_s = src[order].astype(np.int64)
    dst_s = dst[order].astype(np.int64)
    rel_s = rel_id[order].astype(np.int64)

    def wrap16(flat):
        # slot j -> [j % 16, j // 16], replicated 8x down the partitions
        w = flat.reshape(totslot // 16, 16).T          # [16, cols]
        return np.tile(w, (8, 1)).copy()               # [128, cols]

    in_maps = []
    for c in range(ncores):
        m = core_sorted == c
        slots = slot_sorted[m]
        srcf = np.zeros(totslot, np.int16)
        relf = np.zeros(totslot, np.int16)
        dstf = np.zeros(totslot, np.int16)
        dlf = np.full(totslot, -1.0, np.float32)
        sc = src_s[m]
        srcf[slots] = np.where(sc >= split, sc - split, sc).astype(np.int16)
        relf[slots] = rel_s[m].astype(np.int16)
        dloc = dst_s[m] - c * ns
        dstf[slots] = dloc.astype(np.int16)
        dlf[slots] = (dloc - (dloc // blk) * blk).astype(np.float32)

        in_maps.append({
            "ent": ent,
            "entloc": np.ascontiguousarray(ent[c * ns:(c + 1) * ns]),
            "rel": rel_emb,
            "w": neigh_w,
            "iota": np.broadcast_to(
                np.arange(blk, dtype=np.float32), (P, blk)).copy(),
            "srcidx": wrap16(srcf),
            "relidx": wrap16(relf),
            "dstidx": wrap16(dstf),
            "dstlf": dlf.reshape(c_total, 128).T.copy(),
        })

    meta = dict(capL=capL, capH=capH, n_chunks=n_chunks,
                blockstart=blockstart, c_total=c_total, totslot=totslot,
                nblk=nblk, ns=ns, n=n, d=d, split=split)
    return meta, in_maps


def build_program(meta, repeat=1):
    n, d, ns, nblk = meta["n"], meta["d"], meta["ns"], meta["nblk"]
    split = meta["split"]
    capL, capH, n_chunks = meta["capL"], meta["capH"], meta["n_chunks"]
    blockstart = meta["blockstart"]
    c_total, totslot = meta["c_total"], meta["totslot"]
    blk = BLK

    nc = bacc.Bacc("TRN2", target_bir_lowering=False, debug=False)
    ent_d = nc.dram_tensor("ent", [n, d], f32, kind="ExternalInput")
    entloc_d = nc.dram_tensor("entloc", [ns, d], f32, kind="ExternalInput")
    rel_d = nc.dram_tensor("rel", [R2, d], f32, kind="ExternalInput")
    w_d = nc.dram_tensor("w", [d, d], f32, kind="ExternalInput")
    iota_d = nc.dram_tensor("iota", [P, blk], f32, kind="ExternalInput")
    srcidx_d = nc.dram_tensor("srcidx", [P, totslot // 16], i16, kind="ExternalInput")
    relidx_d = nc.dram_tensor("relidx", [P, totslot // 16], i16, kind="ExternalInput")
    dstidx_d = nc.dram_tensor("dstidx", [P, totslot // 16], i16, kind="ExternalInput")
    dstlf_d = nc.dram_tensor("dstlf", [P, c_total], f32, kind="ExternalInput")
    outT_d = nc.dram_tensor("outT", [d, ns], f32, kind="ExternalOutput")

    nsup = nblk // SB

    with tile.TileContext(nc) as tc:
        with (
            tc.tile_pool(name="const", bufs=1) as cp,
            tc.tile_pool(name="sup", bufs=2) as supp,
            tc.tile_pool(name="small", bufs=4) as smp,
            tc.tile_pool(name="epi", bufs=2) as epp,
            tc.tile_pool(name="psum", bufs=2, space="PSUM") as psp,
            tc.tile_pool(name="episum", bufs=2, space="PSUM") as epsp,
        ):
            iota_t = cp.tile([P, blk], f32)
            nc.sync.dma_start(out=iota_t[:], in_=iota_d[:])
            w_t = cp.tile([d, d], f32)
            nc.sync.dma_start(out=w_t[:], in_=w_d[:])
            ident = cp.tile([P, P], f32)
            make_identity(nc, ident[:])

            def body(_iv=None):
                for s in range(nsup):
                    b0 = s * SB
                    blks = [b for b in range(b0, b0 + SB)]
                    ss0 = int(blockstart[b0])
                    ss1 = int(blockstart[b0 + SB])
                    nch = (ss1 - ss0) // 128
                    c0 = ss0 // 128
                    col0, col1 = ss0 // 16, ss1 // 16

                    srcw = supp.tile([P, col1 - col0], i16, tag="srcw")
                    relw = supp.tile([P, col1 - col0], i16, tag="relw")
                    dstw = supp.tile([P, col1 - col0], i16, tag="dstw")
                    dlf = supp.tile([P, nch], f32, tag="dlf")
                    nc.sync.dma_start(out=srcw[:], in_=srcidx_d[:, col0:col1])
                    nc.sync.dma_start(out=relw[:], in_=relidx_d[:, col0:col1])
                    nc.sync.dma_start(out=dstw[:], in_=dstidx_d[:, col0:col1])
                    nc.sync.dma_start(out=dlf[:], in_=dstlf_d[:, c0:c0 + nch])

                    hs = supp.tile([P, nch * d], f32, tag="hs")
                    rl = supp.tile([P, nch * d], f32, tag="rl")
                    hd = supp.tile([P, nch * d], f32, tag="hd")

                    for b in blks:
                        for hi, cap, base in ((0, int(capL[b]), 0),
                                              (1, int(capH[b]), split)):
                            if cap == 0:
                                continue
                            sec0 = int(blockstart[b]) + (int(capL[b]) if hi else 0)
                            lo = sec0 - ss0          # slot offset in super
                            nc.gpsimd.dma_gather(
                                out_ap=hs[:, lo:lo + cap]
                                    .rearrange("p (c x) -> p c x", x=d),
                                in_ap=ent_d[base:, :] if base else ent_d[:],
                                idxs_ap=srcw[:, lo // 16:(lo + cap) // 16],
                                num_idxs=cap,
                                num_idxs_reg=cap,
                                elem_size=d,
                                single_packet=False,
                            )
                    nc.gpsimd.dma_gather(
                        out_ap=rl[:].rearrange("p (c x) -> p c x", x=d),
                        in_ap=rel_d[:],
                        idxs_ap=relw[:],
                        num_idxs=ss1 - ss0,
                        num_idxs_reg=ss1 - ss0,
                        elem_size=d,
                        single_packet=False,
                    )
                    nc.gpsimd.dma_gather(
                        out_ap=hd[:].rearrange("p (c x) -> p c x", x=d),
                        in_ap=entloc_d[:],
                        idxs_ap=dstw[:],
                        num_idxs=ss1 - ss0,
                        num_idxs_reg=ss1 - ss0,
                        elem_size=d,
                        single_packet=False,
                    )

                    compp = supp.tile([P, nch * (d + 1)], f32, tag="compp")
                    compp3 = compp[:].rearrange("p (c x) -> p c x", x=d + 1)
                    nc.vector.memset(compp3[:, :, d:d + 1], 1.0)
                    nc.vector.tensor_tensor(
                        out=compp3[:, :, :d],
                        in0=hs[:].rearrange("p (c x) -> p c x", x=d),
                        in1=rl[:].rearrange("p (c x) -> p c x", x=d),
                        op=mybir.AluOpType.mult,
                    )

                    ci = c0
                    for b in blks:
                        nch_b = int(n_chunks[b])
                        psum_cur = psp.tile([blk, d + 1], f32, tag="acc")
                        for k in range(nch_b):
                            i = ci - c0
                            comp_i = compp[:, i * (d + 1):i * (d + 1) + d]
                            rhs_i = compp[:, i * (d + 1):(i + 1) * (d + 1)]
                            scratch = smp.tile([P, d], f32, tag="scratch")
                            score = smp.tile([P, 1], f32, tag="score")
                            nc.vector.scalar_tensor_tensor(
                                out=scratch[:], in0=comp_i, scalar=1.0,
                                in1=hd[:, i * d:(i + 1) * d],
                                op0=mybir.AluOpType.mult,
                                op1=mybir.AluOpType.mult,
                                accum_out=score[:],
                            )
                            ex = smp.tile([P, 1], f32, tag="ex")
                            nc.scalar.activation(
                                out=ex[:], in_=score[:],
                                func=mybir.ActivationFunctionType.Exp)
                            A = smp.tile([P, blk], f32, tag="A")
                            nc.vector.tensor_scalar(
                                out=A[:], in0=iota_t[:],
                                scalar1=dlf[:, i:i + 1], scalar2=ex[:],
                                op0=mybir.AluOpType.is_equal,
                                op1=mybir.AluOpType.mult,
                            )
                            nc.tensor.matmul(
                                out=psum_cur[:], lhsT=A[:], rhs=rhs_i,
                                start=(k == 0), stop=(k == nch_b - 1))
                            ci += 1
                        # block epilogue
                        den = epp.tile([blk, 1], f32, tag="den")
                        nc.vector.tensor_scalar_add(
                            out=den[:], in0=psum_cur[:, d:d + 1], scalar1=1e-37)
                        rinv = epp.tile([blk, 1], f32, tag="rinv")
                        nc.vector.reciprocal(out=rinv[:], in_=den[:])
                        nb = epp.tile([blk, d], f32, tag="nb")
                        nc.vector.tensor_scalar_mul(
                            out=nb[:], in0=psum_cur[:, :d], scalar1=rinv[:])
                        nT_ps = epsp.tile([d, blk], f32, tag="nT")
                        nc.tensor.transpose(
                            out=nT_ps[:], in_=nb[:], identity=ident[:blk, :blk])
                        nT = epp.tile([d, blk], f32, tag="nTs")
                        nc.vector.tensor_copy(out=nT[:], in_=nT_ps[:])
                        oT_ps = epsp.tile([d, blk], f32, tag="oT")
                        nc.tensor.matmul(out=oT_ps[:], lhsT=w_t[:], rhs=nT[:],
                                         start=True, stop=True)
                        ob = epp.tile([d, blk], f32, tag="ob")
                        nc.scalar.activation(
                            out=ob[:], in_=oT_ps[:],
                            func=mybir.ActivationFunctionType.Tanh)
                        nc.sync.dma_start(
                            out=outT_d[:, b * blk:(b + 1) * blk], in_=ob[:])

            if repeat == 1:
                body()
            else:
                with tc.For_i(0, repeat, 1) as iv:
                    body(iv)

    nc.compile()
    return nc


_CACHE = {}


def _get_compiled(meta):
    key = (meta["c_total"], tuple(meta["n_chunks"]), tuple(meta["capL"]),
           meta["n"], meta["d"])
    if key not in _CACHE:
        _CACHE[key] = build_program(meta)
    return _CACHE[key]


def kernel(ent_emb, rel_emb, neigh_w, node_id, src, dst, rel_id):
    ent_emb = np.asarray(ent_emb, dtype=np.float32)
    rel_emb = np.asarray(rel_emb, dtype=np.float32)
    neigh_w = np.asarray(neigh_w, dtype=np.float32)
    node_id = np.asarray(node_id, dtype=np.int32)
    src = np.asarray(src, dtype=np.int32)
    dst = np.asarray(dst, dtype=np.int32)
    rel_id = np.asarray(rel_id, dtype=np.int32)

    ent = np.ascontiguousarray(ent_emb[node_id])   # node features (arange id)
    meta, in_maps = build_layout(ent, rel_emb, neigh_w, src, dst, rel_id)
    nc = _get_compiled(meta)
    res = run_bass_kernel_spmd(nc, in_maps, core_ids=list(range(NCORES)))
    out = np.concatenate([res.results[c]["outT"].T for c in range(NCORES)], axis=0)
    return out.astype(np.float32)



# revision 3
# speedup vs baseline: 1.8237x; 1.8237x over previous
"""Trainium2 Bass kernel for nn_CompLayer_37512244363763 (GNN message passing).

Strategy: dst-range sharding over 8 cores (no collectives). Each core owns
nodes [c*5000, (c+1)*5000). Host pre-sorts edges by (dst-block, src-range),
pads each (block, src-half) section to 128-edge chunks with shared-across-core
capacities so one SPMD program serves all cores.

Per 128-edge chunk (all edges target one 125-node block):
  hs = ent[src], rl = rel_emb[rel_id], hd = ent[dst]   (dma_gather, int16 idx)
  comp = hs*rl ; score = rowsum(comp*hd) ; ex = exp(score)
  A[e,n] = (dst_local[e]==n) * ex[e]                    (one tensor_scalar op)
  psum[125,129] += A^T @ [comp | 1]                     (PE accumulate)
Block epilogue: neigh = psum[:,:128]/(psum[:,128]+eps); out = tanh(neigh @ W)
computed transposed so the store needs no DMA transpose; host transposes back.
"""
import numpy as np

import concourse.bacc as bacc
import concourse.bass as bass
import concourse.tile as tile
import concourse.mybir as mybir
from concourse.bass_utils import run_bass_kernel_spmd
from concourse.masks import make_identity

f32 = mybir.dt.float32
i16 = mybir.dt.int16

N = 40000
E = 640000
D = 128
R2 = 474
NCORES = 8
NS = N // NCORES          # nodes per core
BLK = 125                 # nodes per PSUM block
NBLK = NS // BLK          # blocks per core
P = 128                   # edges per chunk
SB = 2                    # blocks per gather super
SPLIT = 32768             # int16 index limit for src gathers


def _ceil128(x):
    return ((x + 127) // 128) * 128


def build_layout(ent, rel_emb, neigh_w, src, dst, rel_id, split=SPLIT,
                 n=N, d=D, ncores=NCORES, blk=BLK):
    """Host-side shard + sort + pad. Returns (meta, in_maps)."""
    ns = n // ncores
    nblk = ns // blk
    e = src.shape[0]

    gb = dst.astype(np.int64) // blk                   # global block id
    half = (src >= split).astype(np.int64)
    key = gb * 2 + half
    order = np.argsort(key, kind="stable")
    nsec_g = ncores * nblk * 2
    cnt = np.bincount(key, minlength=nsec_g)
    Lc = cnt[0::2].reshape(ncores, nblk)
    Hc = cnt[1::2].reshape(ncores, nblk)
    capL = _ceil128(Lc.max(axis=0))
    capH = _ceil128(Hc.max(axis=0))
    empty = (capL + capH) == 0
    capL[empty] = 128
    n_chunks = (capL + capH) // 128                    # per in-core block
    c_total = int(n_chunks.sum())
    totslot = 128 * c_total

    blockstart = np.zeros(nblk + 1, np.int64)
    blockstart[1:] = np.cumsum(capL + capH)
    # per-core-local section starts, indexed by (b_incore*2 + half)
    secstart = np.zeros(nblk * 2, np.int64)
    secstart[0::2] = blockstart[:-1]
    secstart[1::2] = blockstart[:-1] + capL

    # per-edge slot within its core's slot space
    gfirst = np.zeros(nsec_g, np.int64)
    gfirst[1:] = np.cumsum(cnt)[:-1]
    ranks = np.arange(e, dtype=np.int64) - np.repeat(gfirst, cnt)
    key_sorted = key[order]
    sec_local = (key_sorted % (nblk * 2))
    slot_sorted = secstart[sec_local] + ranks
    core_sorted = key_sorted // (nblk * 2)

    src_s = src[order].astype(np.int64)
    dst_s = dst[order].astype(np.int64)
    rel_s = rel_id[order].astype(np.int64)

    def wrap16(flat):
        # slot j -> [j % 16, j // 16], replicated 8x down the partitions
        w = flat.reshape(totslot // 16, 16).T          # [16, cols]
        return np.tile(w, (8, 1)).copy()               # [128, cols]

    in_maps = []
    for c in range(ncores):
        m = core_sorted == c
        slots = slot_sorted[m]
        srcf = np.zeros(totslot, np.int16)
        relf = np.zeros(totslot, np.int16)
        dstf = np.zeros(totslot, np.int16)
        dlf = np.full(totslot, -1.0, np.float32)
        sc = src_s[m]
        srcf[slots] = np.where(sc >= split, sc - split, sc).astype(np.int16)
        relf[slots] = rel_s[m].astype(np.int16)
        dloc = dst_s[m] - c * ns
        dstf[slots] = dloc.astype(np.int16)
        dlf[slots] = (dloc - (dloc // blk) * blk).astype(np.float32)

        in_maps.append({
            "ent": ent,
            "entloc": np.ascontiguousarray(ent[c * ns:(c + 1) * ns]),
            "rel": rel_emb,
            "w": neigh_w,
            "iota": np.broadcast_to(
                np.arange(blk, dtype=np.float32), (P, blk)).copy(),
            "srcidx": wrap16(srcf),
            "relidx": wrap16(relf),
            "dstidx": wrap16(dstf),
            "dstlf": dlf.reshape(c_total, 128).T.copy(),
        })

    meta = dict(capL=capL, capH=capH, n_chunks=n_chunks,
                blockstart=blockstart, c_total=c_total, totslot=totslot,
                nblk=nblk, ns=ns, n=n, d=d, split=split)
    return meta, in_maps


def build_program(meta, repeat=1):
    n, d, ns, nblk = meta["n"], meta["d"], meta["ns"], meta["nblk"]
    split = meta["split"]
    capL, capH, n_chunks = meta["capL"], meta["capH"], meta["n_chunks"]
    blockstart = meta["blockstart"]
    c_total, totslot = meta["c_total"], meta["totslot"]
    blk = BLK

    nc = bacc.Bacc("TRN2", target_bir_lowering=False, debug=False,
                   num_swdge_queues=4)
    ent_d = nc.dram_tensor("ent", [n, d], f32, kind="ExternalInput")
    entloc_d = nc.dram_tensor("entloc", [ns, d], f32, kind="ExternalInput")
    rel_d = nc.dram_tensor("rel", [R2, d], f32, kind="ExternalInput")
    w_d = nc.dram_tensor("w", [d, d], f32, kind="ExternalInput")
    iota_d = nc.dram_tensor("iota", [P, blk], f32, kind="ExternalInput")
    srcidx_d = nc.dram_tensor("srcidx", [P, totslot // 16], i16, kind="ExternalInput")
    relidx_d = nc.dram_tensor("relidx", [P, totslot // 16], i16, kind="ExternalInput")
    dstidx_d = nc.dram_tensor("dstidx", [P, totslot // 16], i16, kind="ExternalInput")
    dstlf_d = nc.dram_tensor("dstlf", [P, c_total], f32, kind="ExternalInput")
    outT_d = nc.dram_tensor("outT", [d, ns], f32, kind="ExternalOutput")

    nsup = nblk // SB

    with tile.TileContext(nc) as tc:
        with (
            tc.tile_pool(name="const", bufs=1) as cp,
            tc.tile_pool(name="sup", bufs=2) as supp,
            tc.tile_pool(name="small", bufs=4) as smp,
            tc.tile_pool(name="epi", bufs=2) as epp,
            tc.tile_pool(name="psum", bufs=2, space="PSUM") as psp,
            tc.tile_pool(name="episum", bufs=2, space="PSUM") as epsp,
        ):
            iota_t = cp.tile([P, blk], f32)
            nc.sync.dma_start(out=iota_t[:], in_=iota_d[:])
            w_t = cp.tile([d, d], f32)
            nc.sync.dma_start(out=w_t[:], in_=w_d[:])
            ident = cp.tile([P, P], f32)
            make_identity(nc, ident[:])

            def body(_iv=None):
                for s in range(nsup):
                    b0 = s * SB
                    blks = [b for b in range(b0, b0 + SB)]
                    ss0 = int(blockstart[b0])
                    ss1 = int(blockstart[b0 + SB])
                    nch = (ss1 - ss0) // 128
                    c0 = ss0 // 128
                    col0, col1 = ss0 // 16, ss1 // 16

                    srcw = supp.tile([P, col1 - col0], i16, tag="srcw")
                    relw = supp.tile([P, col1 - col0], i16, tag="relw")
                    dstw = supp.tile([P, col1 - col0], i16, tag="dstw")
                    dlf = supp.tile([P, nch], f32, tag="dlf")
                    nc.sync.dma_start(out=srcw[:], in_=srcidx_d[:, col0:col1])
                    nc.sync.dma_start(out=relw[:], in_=relidx_d[:, col0:col1])
                    nc.sync.dma_start(out=dstw[:], in_=dstidx_d[:, col0:col1])
                    nc.sync.dma_start(out=dlf[:], in_=dstlf_d[:, c0:c0 + nch])

                    hs = supp.tile([P, nch * d], f32, tag="hs")
                    rl = supp.tile([P, nch * d], f32, tag="rl")
                    hd = supp.tile([P, nch * d], f32, tag="hd")

                    qn = 0
                    for b in blks:
                        for hi, cap, base in ((0, int(capL[b]), 0),
                                              (1, int(capH[b]), split)):
                            if cap == 0:
                                continue
                            sec0 = int(blockstart[b]) + (int(capL[b]) if hi else 0)
                            lo = sec0 - ss0          # slot offset in super
                            nc.gpsimd.dma_gather(
                                out_ap=hs[:, lo:lo + cap]
                                    .rearrange("p (c x) -> p c x", x=d),
                                in_ap=ent_d[base:, :] if base else ent_d[:],
                                idxs_ap=srcw[:, lo // 16:(lo + cap) // 16],
                                num_idxs=cap,
                                num_idxs_reg=cap,
                                elem_size=d,
                                single_packet=False,
                                queue_num=qn % 4,
                            )
                            qn += 1
                    # split the big rel/dst gathers in halves across queues
                    nsl = ss1 - ss0
                    half_ch = (nsl // 128) // 2
                    pieces = [(0, half_ch * 128), (half_ch * 128, nsl)]
                    for lo, hi2 in pieces:
                        nc.gpsimd.dma_gather(
                            out_ap=rl[:, lo:hi2]
                                .rearrange("p (c x) -> p c x", x=d),
                            in_ap=rel_d[:],
                            idxs_ap=relw[:, lo // 16:hi2 // 16],
                            num_idxs=hi2 - lo,
                            num_idxs_reg=hi2 - lo,
                            elem_size=d,
                            single_packet=False,
                            queue_num=qn % 4,
                        )
                        qn += 1
                    for lo, hi2 in pieces:
                        nc.gpsimd.dma_gather(
                            out_ap=hd[:, lo:hi2]
                                .rearrange("p (c x) -> p c x", x=d),
                            in_ap=entloc_d[:],
                            idxs_ap=dstw[:, lo // 16:hi2 // 16],
                            num_idxs=hi2 - lo,
                            num_idxs_reg=hi2 - lo,
                            elem_size=d,
                            single_packet=False,
                            queue_num=qn % 4,
                        )
                        qn += 1

                    compp = supp.tile([P, nch * (d + 1)], f32, tag="compp")
                    compp3 = compp[:].rearrange("p (c x) -> p c x", x=d + 1)
                    nc.vector.memset(compp3[:, :, d:d + 1], 1.0)
                    nc.vector.tensor_tensor(
                        out=compp3[:, :, :d],
                        in0=hs[:].rearrange("p (c x) -> p c x", x=d),
                        in1=rl[:].rearrange("p (c x) -> p c x", x=d),
                        op=mybir.AluOpType.mult,
                    )

                    ci = c0
                    for b in blks:
                        nch_b = int(n_chunks[b])
                        psum_cur = psp.tile([blk, d + 1], f32, tag="acc")
                        for k in range(nch_b):
                            i = ci - c0
                            comp_i = compp[:, i * (d + 1):i * (d + 1) + d]
                            rhs_i = compp[:, i * (d + 1):(i + 1) * (d + 1)]
                            scratch = smp.tile([P, d], f32, tag="scratch")
                            score = smp.tile([P, 1], f32, tag="score")
                            nc.vector.scalar_tensor_tensor(
                                out=scratch[:], in0=comp_i, scalar=1.0,
                                in1=hd[:, i * d:(i + 1) * d],
                                op0=mybir.AluOpType.mult,
                                op1=mybir.AluOpType.mult,
                                accum_out=score[:],
                            )
                            ex = smp.tile([P, 1], f32, tag="ex")
                            nc.scalar.activation(
                                out=ex[:], in_=score[:],
                                func=mybir.ActivationFunctionType.Exp)
                            A = smp.tile([P, blk], f32, tag="A")
                            nc.vector.tensor_scalar(
                                out=A[:], in0=iota_t[:],
                                scalar1=dlf[:, i:i + 1], scalar2=ex[:],
                                op0=mybir.AluOpType.is_equal,
                                op1=mybir.AluOpType.mult,
                            )
                            nc.tensor.matmul(
                                out=psum_cur[:], lhsT=A[:], rhs=rhs_i,
                                start=(k == 0), stop=(k == nch_b - 1))
                            ci += 1
                        # block epilogue
                        den = epp.tile([blk, 1], f32, tag="den")
                        nc.vector.tensor_scalar_add(
                            out=den[:], in0=psum_cur[:, d:d + 1], scalar1=1e-37)
                        rinv = epp.tile([blk, 1], f32, tag="rinv")
                        nc.vector.reciprocal(out=rinv[:], in_=den[:])
                        nb = epp.tile([blk, d], f32, tag="nb")
                        nc.vector.tensor_scalar_mul(
                            out=nb[:], in0=psum_cur[:, :d], scalar1=rinv[:])
                        nT_ps = epsp.tile([d, blk], f32, tag="nT")
                        nc.tensor.transpose(
                            out=nT_ps[:], in_=nb[:], identity=ident[:blk, :blk])
                        nT = epp.tile([d, blk], f32, tag="nTs")
                        nc.vector.tensor_copy(out=nT[:], in_=nT_ps[:])
                        oT_ps = epsp.tile([d, blk], f32, tag="oT")
                        nc.tensor.matmul(out=oT_ps[:], lhsT=w_t[:], rhs=nT[:],
                                         start=True, stop=True)
                        ob = epp.tile([d, blk], f32, tag="ob")
                        nc.scalar.activation(
                            out=ob[:], in_=oT_ps[:],
                            func=mybir.ActivationFunctionType.Tanh)
                        nc.sync.dma_start(
                            out=outT_d[:, b * blk:(b + 1) * blk], in_=ob[:])

            if repeat == 1:
                body()
            else:
                with tc.For_i(0, repeat, 1) as iv:
                    body(iv)

    nc.compile()
    return nc


_CACHE = {}


def _get_compiled(meta):
    key = (meta["c_total"], tuple(meta["n_chunks"]), tuple(meta["capL"]),
           meta["n"], meta["d"])
    if key not in _CACHE:
        _CACHE[key] = build_program(meta)
    return _CACHE[key]


def kernel(ent_emb, rel_emb, neigh_w, node_id, src, dst, rel_id):
    ent_emb = np.asarray(ent_emb, dtype=np.float32)
    rel_emb = np.asarray(rel_emb, dtype=np.float32)
    neigh_w = np.asarray(neigh_w, dtype=np.float32)
    node_id = np.asarray(node_id, dtype=np.int32)
    src = np.asarray(src, dtype=np.int32)
    dst = np.asarray(dst, dtype=np.int32)
    rel_id = np.asarray(rel_id, dtype=np.int32)

    ent = np.ascontiguousarray(ent_emb[node_id])   # node features (arange id)
    meta, in_maps = build_layout(ent, rel_emb, neigh_w, src, dst, rel_id)
    nc = _get_compiled(meta)
    res = run_bass_kernel_spmd(nc, in_maps, core_ids=list(range(NCORES)))
    out = np.concatenate([res.results[c]["outT"].T for c in range(NCORES)], axis=0)
    return out.astype(np.float32)



# revision 12
# speedup vs baseline: 1.8480x; 1.0133x over previous
"""Trainium2 Bass kernel for nn_CompLayer_37512244363763 (GNN message passing).

Strategy: dst-range sharding over 8 cores (no collectives). Each core owns
nodes [c*5000, (c+1)*5000). Host pre-sorts edges by (dst-block, src-range),
pads each (block, src-half) section to 128-edge chunks with shared-across-core
capacities so one SPMD program serves all cores.

Per 128-edge chunk (all edges target one 125-node block):
  hs = ent[src], rl = rel_emb[rel_id], hd = ent[dst]   (dma_gather, int16 idx)
  comp = hs*rl ; score = rowsum(comp*hd) ; ex = exp(score)
  A[e,n] = (dst_local[e]==n) * ex[e]                    (one tensor_scalar op)
  psum[125,129] += A^T @ [comp | 1]                     (PE accumulate)
Block epilogue: neigh = psum[:,:128]/(psum[:,128]+eps); out = tanh(neigh @ W)
computed transposed so the store needs no DMA transpose; host transposes back.
"""
import numpy as np

import concourse.bacc as bacc
import concourse.bass as bass
import concourse.tile as tile
import concourse.mybir as mybir
from concourse.bass_utils import run_bass_kernel_spmd
from concourse.masks import make_identity

f32 = mybir.dt.float32
f16 = mybir.dt.float16
i16 = mybir.dt.int16

N = 40000
E = 640000
D = 128
R2 = 474
NCORES = 8
NS = N // NCORES          # nodes per core
BLK = 125                 # nodes per PSUM block
NBLK = NS // BLK          # blocks per core
P = 128                   # edges per chunk
SB = 2                    # blocks per gather super
SPLIT = 32768             # int16 index limit for src gathers


def _ceil128(x):
    return ((x + 127) // 128) * 128


def build_layout(ent, rel_emb, neigh_w, src, dst, rel_id, split=SPLIT,
                 n=N, d=D, ncores=NCORES, blk=BLK):
    """Host-side shard + sort + pad. Returns (meta, in_maps)."""
    ns = n // ncores
    nblk = ns // blk
    e = src.shape[0]

    gb = dst.astype(np.int64) // blk                   # global block id
    half = (src >= split).astype(np.int64)
    key = gb * 2 + half
    order = np.argsort(key, kind="stable")
    nsec_g = ncores * nblk * 2
    cnt = np.bincount(key, minlength=nsec_g)
    Lc = cnt[0::2].reshape(ncores, nblk)
    Hc = cnt[1::2].reshape(ncores, nblk)
    capL = _ceil128(Lc.max(axis=0))
    capH = _ceil128(Hc.max(axis=0))
    empty = (capL + capH) == 0
    capL[empty] = 128
    n_chunks = (capL + capH) // 128                    # per in-core block
    c_total = int(n_chunks.sum())
    totslot = 128 * c_total

    blockstart = np.zeros(nblk + 1, np.int64)
    blockstart[1:] = np.cumsum(capL + capH)
    # per-core-local section starts, indexed by (b_incore*2 + half)
    secstart = np.zeros(nblk * 2, np.int64)
    secstart[0::2] = blockstart[:-1]
    secstart[1::2] = blockstart[:-1] + capL

    # per-edge slot within its core's slot space
    gfirst = np.zeros(nsec_g, np.int64)
    gfirst[1:] = np.cumsum(cnt)[:-1]
    ranks = np.arange(e, dtype=np.int64) - np.repeat(gfirst, cnt)
    key_sorted = key[order]
    sec_local = (key_sorted % (nblk * 2))
    slot_sorted = secstart[sec_local] + ranks
    core_sorted = key_sorted // (nblk * 2)

    src_s = src[order].astype(np.int64)
    dst_s = dst[order].astype(np.int64)
    rel_s = rel_id[order].astype(np.int64)

    def wrap16(flat):
        # slot j -> [j % 16, j // 16], replicated 8x down the partitions
        w = flat.reshape(totslot // 16, 16).T          # [16, cols]
        return np.tile(w, (8, 1)).copy()               # [128, cols]

    in_maps = []
    for c in range(ncores):
        m = core_sorted == c
        slots = slot_sorted[m]
        srcf = np.zeros(totslot, np.int16)
        relf = np.zeros(totslot, np.int16)
        dstf = np.zeros(totslot, np.int16)
        dlf = np.full(totslot, -1.0, np.float32)
        sc = src_s[m]
        srcf[slots] = np.where(sc >= split, sc - split, sc).astype(np.int16)
        relf[slots] = rel_s[m].astype(np.int16)
        dloc = dst_s[m] - c * ns
        dstf[slots] = dloc.astype(np.int16)
        dlf[slots] = (dloc - (dloc // blk) * blk).astype(np.float32)

        in_maps.append({
            "ent": ent,
            "entloc": np.ascontiguousarray(ent[c * ns:(c + 1) * ns]),
            "rel": rel_emb,
            "w": neigh_w,
            "iota": np.broadcast_to(
                np.arange(blk, dtype=np.float32), (P, blk)).copy(),
            "srcidx": wrap16(srcf),
            "relidx": wrap16(relf),
            "dstidx": wrap16(dstf),
            "dstlf": dlf.reshape(c_total, 128).T.copy(),
        })

    meta = dict(capL=capL, capH=capH, n_chunks=n_chunks,
                blockstart=blockstart, c_total=c_total, totslot=totslot,
                nblk=nblk, ns=ns, n=n, d=d, split=split)
    return meta, in_maps


def build_program(meta, repeat=1):
    n, d, ns, nblk = meta["n"], meta["d"], meta["ns"], meta["nblk"]
    split = meta["split"]
    capL, capH, n_chunks = meta["capL"], meta["capH"], meta["n_chunks"]
    blockstart = meta["blockstart"]
    c_total, totslot = meta["c_total"], meta["totslot"]
    blk = BLK

    nc = bacc.Bacc("TRN2", target_bir_lowering=False, debug=False,
                   num_swdge_queues=4)
    ent_d = nc.dram_tensor("ent", [n, d], f32, kind="ExternalInput")
    entloc_d = nc.dram_tensor("entloc", [ns, d], f32, kind="ExternalInput")
    rel_d = nc.dram_tensor("rel", [R2, d], f32, kind="ExternalInput")
    w_d = nc.dram_tensor("w", [d, d], f32, kind="ExternalInput")
    iota_d = nc.dram_tensor("iota", [P, blk], f32, kind="ExternalInput")
    srcidx_d = nc.dram_tensor("srcidx", [P, totslot // 16], i16, kind="ExternalInput")
    relidx_d = nc.dram_tensor("relidx", [P, totslot // 16], i16, kind="ExternalInput")
    dstidx_d = nc.dram_tensor("dstidx", [P, totslot // 16], i16, kind="ExternalInput")
    dstlf_d = nc.dram_tensor("dstlf", [P, c_total], f32, kind="ExternalInput")
    outT_d = nc.dram_tensor("outT", [d, ns], f32, kind="ExternalOutput")

    nsup = nblk // SB

    with tile.TileContext(nc) as tc:
        with (
            tc.tile_pool(name="const", bufs=1) as cp,
            tc.tile_pool(name="sup", bufs=2) as supp,
            tc.tile_pool(name="small", bufs=4) as smp,
            tc.tile_pool(name="epi", bufs=2) as epp,
            tc.tile_pool(name="psum", bufs=2, space="PSUM") as psp,
            tc.tile_pool(name="episum", bufs=2, space="PSUM") as epsp,
        ):
            iota_t = cp.tile([P, blk], f32)
            nc.sync.dma_start(out=iota_t[:], in_=iota_d[:])
            w_t = cp.tile([d, d], f32)
            nc.sync.dma_start(out=w_t[:], in_=w_d[:])
            ident = cp.tile([P, P], f32)
            make_identity(nc, ident[:])

            def body(_iv=None):
                for s in range(nsup):
                    b0 = s * SB
                    blks = [b for b in range(b0, b0 + SB)]
                    ss0 = int(blockstart[b0])
                    ss1 = int(blockstart[b0 + SB])
                    nch = (ss1 - ss0) // 128
                    c0 = ss0 // 128
                    col0, col1 = ss0 // 16, ss1 // 16

                    srcw = supp.tile([P, col1 - col0], i16, tag="srcw")
                    relw = supp.tile([P, col1 - col0], i16, tag="relw")
                    dstw = supp.tile([P, col1 - col0], i16, tag="dstw")
                    dlf = supp.tile([P, nch], f32, tag="dlf")
                    nc.sync.dma_start(out=srcw[:], in_=srcidx_d[:, col0:col1])
                    nc.sync.dma_start(out=relw[:], in_=relidx_d[:, col0:col1])
                    nc.sync.dma_start(out=dstw[:], in_=dstidx_d[:, col0:col1])
                    nc.sync.dma_start(out=dlf[:], in_=dstlf_d[:, c0:c0 + nch])

                    hs = supp.tile([P, nch * d], f32, tag="hs")
                    rl = supp.tile([P, nch * d], f32, tag="rl")
                    hd = supp.tile([P, nch * d], f32, tag="hd")

                    qn = 0
                    for b in blks:
                        for hi, cap, base in ((0, int(capL[b]), 0),
                                              (1, int(capH[b]), split)):
                            if cap == 0:
                                continue
                            sec0 = int(blockstart[b]) + (int(capL[b]) if hi else 0)
                            lo = sec0 - ss0          # slot offset in super
                            nc.gpsimd.dma_gather(
                                out_ap=hs[:, lo:lo + cap]
                                    .rearrange("p (c x) -> p c x", x=d),
                                in_ap=ent_d[base:, :] if base else ent_d[:],
                                idxs_ap=srcw[:, lo // 16:(lo + cap) // 16],
                                num_idxs=cap,
                                num_idxs_reg=cap,
                                elem_size=d,
                                single_packet=False,
                                queue_num=qn % 4,
                            )
                            qn += 1
                    # split the big rel/dst gathers in halves across queues
                    nsl = ss1 - ss0
                    half_ch = (nsl // 128) // 2
                    pieces = [(0, half_ch * 128), (half_ch * 128, nsl)]
                    for lo, hi2 in pieces:
                        nc.gpsimd.dma_gather(
                            out_ap=rl[:, lo:hi2]
                                .rearrange("p (c x) -> p c x", x=d),
                            in_ap=rel_d[:],
                            idxs_ap=relw[:, lo // 16:hi2 // 16],
                            num_idxs=hi2 - lo,
                            num_idxs_reg=hi2 - lo,
                            elem_size=d,
                            single_packet=False,
                            queue_num=qn % 4,
                        )
                        qn += 1
                    for lo, hi2 in pieces:
                        nc.gpsimd.dma_gather(
                            out_ap=hd[:, lo:hi2]
                                .rearrange("p (c x) -> p c x", x=d),
                            in_ap=entloc_d[:],
                            idxs_ap=dstw[:, lo // 16:hi2 // 16],
                            num_idxs=hi2 - lo,
                            num_idxs_reg=hi2 - lo,
                            elem_size=d,
                            single_packet=False,
                            queue_num=qn % 4,
                        )
                        qn += 1

                    compp = supp.tile([P, nch * (d + 1)], f32, tag="compp")
                    compp3 = compp[:].rearrange("p (c x) -> p c x", x=d + 1)
                    nc.vector.memset(compp3[:, :, d:d + 1], 1.0)
                    nc.vector.tensor_tensor(
                        out=compp3[:, :, :d],
                        in0=hs[:].rearrange("p (c x) -> p c x", x=d),
                        in1=rl[:].rearrange("p (c x) -> p c x", x=d),
                        op=mybir.AluOpType.mult,
                    )

                    ci = c0
                    for b in blks:
                        nch_b = int(n_chunks[b])
                        psum_cur = psp.tile([blk, d + 1], f32, tag="acc")
                        for k in range(nch_b):
                            i = ci - c0
                            comp_i = compp[:, i * (d + 1):i * (d + 1) + d]
                            rhs_i = compp[:, i * (d + 1):(i + 1) * (d + 1)]
                            scratch = smp.tile([P, d], f32, tag="scratch")
                            score = smp.tile([P, 1], f32, tag="score")
                            nc.vector.scalar_tensor_tensor(
                                out=scratch[:], in0=comp_i, scalar=1.0,
                                in1=hd[:, i * d:(i + 1) * d],
                                op0=mybir.AluOpType.mult,
                                op1=mybir.AluOpType.mult,
                                accum_out=score[:],
                            )
                            ex = smp.tile([P, 1], f32, tag="ex")
                            nc.scalar.activation(
                                out=ex[:], in_=score[:],
                                func=mybir.ActivationFunctionType.Exp)
                            A = smp.tile([P, blk], f32, tag="A")
                            nc.vector.tensor_scalar(
                                out=A[:], in0=iota_t[:],
                                scalar1=dlf[:, i:i + 1], scalar2=ex[:],
                                op0=mybir.AluOpType.is_equal,
                                op1=mybir.AluOpType.mult,
                            )
                            nc.tensor.matmul(
                                out=psum_cur[:], lhsT=A[:], rhs=rhs_i,
                                start=(k == 0), stop=(k == nch_b - 1))
                            ci += 1
                        # block epilogue
                        den = epp.tile([blk, 1], f32, tag="den")
                        nc.vector.tensor_scalar_add(
                            out=den[:], in0=psum_cur[:, d:d + 1], scalar1=1e-37)
                        rinv = epp.tile([blk, 1], f32, tag="rinv")
                        nc.vector.reciprocal(out=rinv[:], in_=den[:])
                        nb = epp.tile([blk, d], f32, tag="nb")
                        nc.vector.tensor_scalar_mul(
                            out=nb[:], in0=psum_cur[:, :d], scalar1=rinv[:])
                        nT_ps = epsp.tile([d, blk], f32, tag="nT")
                        nc.tensor.transpose(
                            out=nT_ps[:], in_=nb[:], identity=ident[:blk, :blk])
                        nT = epp.tile([d, blk], f32, tag="nTs")
                        nc.vector.tensor_copy(out=nT[:], in_=nT_ps[:])
                        oT_ps = epsp.tile([d, blk], f32, tag="oT")
                        nc.tensor.matmul(out=oT_ps[:], lhsT=w_t[:], rhs=nT[:],
                                         start=True, stop=True)
                        ob = epp.tile([d, blk], f32, tag="ob")
                        nc.scalar.activation(
                            out=ob[:], in_=oT_ps[:],
                            func=mybir.ActivationFunctionType.Tanh)
                        nc.sync.dma_start(
                            out=outT_d[:, b * blk:(b + 1) * blk], in_=ob[:])

            if repeat == 1:
                body()
            else:
                with tc.For_i(0, repeat, 1) as iv:
                    body(iv)

    nc.compile()
    return nc


_CACHE = {}


def _get_compiled(meta):
    key = (meta["c_total"], tuple(meta["n_chunks"]), tuple(meta["capL"]),
           meta["n"], meta["d"])
    if key not in _CACHE:
        _CACHE[key] = build_program(meta)
    return _CACHE[key]


def kernel(ent_emb, rel_emb, neigh_w, node_id, src, dst, rel_id):
    ent_emb = np.asarray(ent_emb, dtype=np.float32)
    rel_emb = np.asarray(rel_emb, dtype=np.float32)
    neigh_w = np.asarray(neigh_w, dtype=np.float32)
    node_id = np.asarray(node_id, dtype=np.int32)
    src = np.asarray(src, dtype=np.int32)
    dst = np.asarray(dst, dtype=np.int32)
    rel_id = np.asarray(rel_id, dtype=np.int32)

    ent = np.ascontiguousarray(ent_emb[node_id])   # node features (arange id)
    meta, in_maps = build_layout(ent, rel_emb, neigh_w, src, dst, rel_id)
    nc = _get_compiled(meta)
    res = run_bass_kernel_spmd(nc, in_maps, core_ids=list(range(NCORES)))
    out = np.concatenate([res.results[c]["outT"].T for c in range(NCORES)], axis=0)
    return out.astype(np.float32)

